# revision 46
# baseline (speedup 1.0000x reference)
"""Trainium2 Bass kernel for nn_NodesToEdges (gnn message passing).

kernel(**inputs) takes FULL inputs, shards edges across 8 NeuronCores,
gathers node rows on-device, computes
  out[e] = 0.5*(W[e]*(xs-xd)) @ M1 + 0.25*(W[e]*(xs+xd)) @ M2
         = (W[e]*xs) @ Ma + (W[e]*xd) @ Mb,   Ma=.5*M1+.25*M2, Mb=.25*M2-.5*M1
and returns the FULL [E, 3, 32] f32 output.

v5 design (fp16 on device):
- Edges are processed in supertiles of TSUB=4096. The per-supertile
  node-row fetch (src+dst) uses the dma_gather ucode (non-transpose
  mode: index ordinal i lands at partition i%128, block i//128).
  Descriptor generation runs on the GPSIMD Q7 cluster at ~8ns/index
  and is the kernel bottleneck, but each gather instruction executes
  on one Q7 core PAIR selected by queue_num — so the fetch is split
  into 4 chunks issued on queues 0..3, which generate descriptors
  concurrently (4x). (The ucode's transposing gather mode would avoid
  the PE transposes below, but concurrent transposing gathers corrupt
  each other in the shared XBAR unit, so gathers run row-major and
  the PE — which has plenty of headroom here — transposes.)
- dma_gather indices are int16; node ids up to 50000 exceed the int16
  range, so the node table is laid out twice in one DRAM tensor and
  the gather base points at row 65536: the ucode's signed index
  addressing maps wrapped-negative ids to the right rows. Node rows
  sit at a 256-byte stride (ucode requirement) but only the real 192
  bytes are gathered. The host guarantees every chunk ends with an
  int16-positive id (the ucode drops trailing-negative indices).
- Per 512-edge group: the W-multiply runs on the DVE against a
  host-pretransposed W tile; the mix is 2 N=512 matmuls with the
  constant mixing matrices stationary, accumulating in PSUM; the
  Activation engine downcasts PSUM into the fp16 output tile. Output
  leaves the device channel-major; the host un-transposes and upcasts.
"""
import os
import sys

for p in ("/opt/trn_rl_repo", "/root/.axon_site/_ro/trn_rl_repo"):
    if os.path.isdir(p) and p not in sys.path:
        sys.path.append(p)
os.environ.setdefault("JAX_PLATFORMS", "axon")

import numpy as np
import concourse.bass as bass
import concourse.bacc as bacc
import concourse.mybir as mybir
from concourse import tile
from concourse.bass_utils import run_bass_kernel_spmd
from concourse.masks import make_identity

F32 = mybir.dt.float32
F16 = mybir.dt.float16
I16 = mybir.dt.int16
P = 128
CH = 96          # 3 vec dims x 32 channels
CHP = 128        # node row padded to 256B gather granularity
NCORES = 8
TSUB = 4096      # edges per supertile
G = TSUB // 512  # 512-edge groups per supertile
NQ = 8           # gather chunks: [s0,d0,s1,d1,...] 2 rounds x 4 queues
NQ_ACTIVE = 4  # distinct SWDGE queues (Q7 core pairs) used
CHUNK = 2 * TSUB // NQ        # ids per queue chunk (host guarantees the
CPAD = CHUNK                  # last id of each chunk is int16-positive)
NIDX = NQ * CPAD              # gathered columns per supertile
BIG = 98304      # wraparound node table rows (3 * 32768)
BASE = 65536     # gather base row inside the wraparound table

TRACE = False
LAST_RESULTS = {}


def _dma_gather_raw(eng, out_ap, in_ap, idxs_ap, num_idxs, elem_size,
                    elem_step, queue_num):
    """bass.dma_gather for the non-transpose HBM-source case, minus the
    256-byte elem_size restriction (that is an XBAR/transpose-mode
    constraint; the row-major ucode path only needs the row STRIDE to be
    a 256-byte multiple). elem_size=96 fp16 gathers the real 192 bytes
    of each 256-byte-strided node row."""
    import concourse.ap_utils as ap_utils
    assert idxs_ap.dtype == I16
    assert in_ap.dtype == out_ap.dtype
    assert ap_utils.ap_is_contiguous(in_ap.ap[1:])
    assert ap_utils.ap_is_contiguous(out_ap.ap[1:])
    assert ap_utils.ap_is_contiguous(idxs_ap.ap[1:])
    assert in_ap.ap[-1][1] == out_ap.ap[-1][1] == elem_size
    assert out_ap.ap[0][1] * out_ap.ap[1][1] == -(-num_idxs // P) * P
    assert in_ap.ap[0][0] == elem_step
    stride_bytes = elem_step * mybir.dt.size(in_ap.dtype)
    _in_ap = eng.lower_ap_dma(in_ap, for_custom_bir_dma=True)
    _idxs_ap = eng.lower_ap(idxs_ap)
    _out_ap = eng.lower_ap(out_ap)
    return eng.add_instruction(
        mybir.InstDMAGatherAnt(
            name=eng.bass.get_next_instruction_name(),
            ins=[*_in_ap, _idxs_ap,
                 eng.lower_val_access(eng.to_reg(num_idxs))],
            outs=[_out_ap],
            transpose=False,
            num_idxs=num_idxs,
            elem_size=elem_size,
            stride_bytes_256=stride_bytes // 256,
            gen_mode=0,
            single_packet=False,
            queue_num=queue_num,
            sbuf_tokens_per_rank=0,
            sbuf_free_dim_per_rank=0,
            sbuf_free_dim_pad_per_rank=0,
            sbuf_byte_offset=0,
        ))


def _build_kernel(NTS, n_real, n_devices=NCORES):
    nc = bacc.Bacc("TRN2", target_bir_lowering=False, debug=False,
                   num_devices=n_devices, num_swdge_queues=NQ_ACTIVE)
    NBLK = NQ // NQ_ACTIVE
    xnb = nc.declare_dram_parameter("xnb", [BIG, CHP], F16, isOutput=False)
    idx = nc.declare_dram_parameter("idx", [NTS, P, NBLK * (CPAD // 16)], I16,
                                    isOutput=False)
    wt = nc.declare_dram_parameter("wt", [NTS, CH, TSUB], F16, isOutput=False)
    mab = nc.declare_dram_parameter("mab", [CH, CH], F16, isOutput=False)
    mbb = nc.declare_dram_parameter("mbb", [CH, CH], F16, isOutput=False)
    out = nc.declare_dram_parameter("out", [NTS, CH, TSUB], F16, isOutput=True)

    with tile.TileContext(nc) as tc:
        with (
            tc.tile_pool(name="const", bufs=1) as cp,
            tc.tile_pool(name="sb", bufs=3) as sb,
            tc.tile_pool(name="uv", bufs=6) as uv,
            tc.tile_pool(name="ps", bufs=2, space="PSUM") as ps,
            tc.tile_pool(name="po", bufs=3, space="PSUM") as po,
        ):
            for t in range(NTS):
                # chunk q's ids live in partitions [32*(q%NQ_ACTIVE), +32),
                # column block q//NQ_ACTIVE
                si = sb.tile([P, NBLK * (CPAD // 16)], I16, tag="si")
                nc.sync.dma_start(out=si[:], in_=idx[t])
                wtt = sb.tile([CH, TSUB], F16, tag="wt")
                nc.sync.dma_start(out=wtt[:], in_=wt[t])

                # 4 concurrent row-major gathers (one per Q7 pair):
                # chunk q, ordinal i -> xg[i%128, (q*BLKS + i//128)*96 : +96]
                CW = (CPAD // P) * CH
                xg = sb.tile([P, NQ * CW], F16, tag="xg")
                W16 = CPAD // 16
                for q in range(NQ):
                    # real edges in this chunk (pad-edge outputs are
                    # discarded on host, so skip gathering them)
                    real = max(0, min(n_real - t * TSUB - (q // 2) * CHUNK,
                                      CHUNK))
                    nidx = -(-real // P) * P
                    if nidx == 0:
                        continue
                    blk = q // NQ_ACTIVE
                    _dma_gather_raw(
                        nc.gpsimd,
                        out_ap=xg[:, q * CW:q * CW + (nidx // P) * CH]
                            .rearrange("p (b n) -> p b n", n=CH),
                        in_ap=xnb[BASE:, :CH],
                        idxs_ap=si[:, blk * W16:(blk + 1) * W16],
                        num_idxs=nidx, elem_size=CH, elem_step=CHP,
                        queue_num=q % NQ_ACTIVE)

                if t == 0:
                    # constants issued after supertile 0's gathers so the
                    # Q7 pipeline starts immediately; they are ready long
                    # before the first transpose needs them
                    ident = cp.tile([P, P], F16)
                    make_identity(nc, ident[:])
                    mab_t = cp.tile([CH, CH], F16)
                    nc.sync.dma_start(out=mab_t[:], in_=mab[:, :])
                    mbb_t = cp.tile([CH, CH], F16)
                    nc.sync.dma_start(out=mbb_t[:], in_=mbb[:, :])

                ot = sb.tile([CH, TSUB], F16, tag="ot")
                for g in range(G):
                    lo, hi = g * 512, (g + 1) * 512
                    # this group's src / dst blocks: groups 0..3 in chunks
                    # 0-1 (src), matching dst in chunks 2-3
                    sq, so = divmod(g * 512, CHUNK)
                    sbase = 2 * sq * CW + (so // P) * CH
                    dbase = (2 * sq + 1) * CW + (so // P) * CH
                    xsT = ps.tile([CH, 512], F16, tag="xsT")
                    xdT = ps.tile([CH, 512], F16, tag="xdT")
                    for j in range(4):
                        nc.tensor.transpose(
                            out=xsT[:, j * P:(j + 1) * P],
                            in_=xg[:, sbase + j * CH:sbase + (j + 1) * CH],
                            identity=ident[:])
                    for j in range(4):
                        nc.tensor.transpose(
                            out=xdT[:, j * P:(j + 1) * P],
                            in_=xg[:, dbase + j * CH:dbase + (j + 1) * CH],
                            identity=ident[:])

                    uT = uv.tile([CH, 512], F16, tag="uT")
                    nc.vector.tensor_tensor(
                        out=uT[:], in0=xsT[:, :], in1=wtt[:, lo:hi],
                        op=mybir.AluOpType.mult)
                    vT = uv.tile([CH, 512], F16, tag="vT")
                    nc.vector.tensor_tensor(
                        out=vT[:], in0=xdT[:, :], in1=wtt[:, lo:hi],
                        op=mybir.AluOpType.mult)

                    oT = po.tile([CH, 512], F32, tag="oT")
                    nc.tensor.matmul(out=oT[:], lhsT=mab_t[:], rhs=uT[:],
                                     start=True, stop=False)
                    nc.tensor.matmul(out=oT[:], lhsT=mbb_t[:], rhs=vT[:],
                                     start=False, stop=True)
                    nc.scalar.copy(out=ot[:, lo:hi], in_=oT[:])

                nc.sync.dma_start(out=out[t], in_=ot[:])

    nc.compile()
    return nc


def _prep_inputs(xn, xe_src, xe_dst, W, M1, M2):
    E = int(xe_src.shape[0])
    nnodes = int(xn.shape[0])
    assert nnodes <= 65536

    src = np.asarray(xe_src).astype(np.int64)
    dst = np.asarray(xe_dst).astype(np.int64)
    W = np.asarray(W, dtype=np.float32)

    EC = -(-E // NCORES)           # edges per core (last may be short)
    ECP = -(-EC // TSUB) * TSUB    # padded per-core edge count
    NTS = ECP // TSUB

    M1d, M2d = np.asarray(M1, np.float64), np.asarray(M2, np.float64)
    Ma = 0.5 * M1d + 0.25 * M2d
    Mb = 0.25 * M2d - 0.5 * M1d
    mab = np.kron(np.eye(3), Ma).astype(np.float16)
    mbb = np.kron(np.eye(3), Mb).astype(np.float16)

    # wraparound node table, rows padded to 128 channels
    xn16 = np.asarray(xn, np.float32).reshape(nnodes, CH).astype(np.float16)
    xnb = np.zeros((BIG, CHP), np.float16)
    n_hi = max(0, nnodes - 32768)
    if n_hi:
        xnb[32768:32768 + n_hi, :CH] = xn16[32768:]
    xnb[BASE:BASE + min(nnodes, 32768), :CH] = xn16[:32768]

    in_maps, spans, perms = [], [], []
    ii = np.arange(CPAD)
    for c in range(NCORES):
        e0, e1 = c * EC, min(E, (c + 1) * EC)
        n = e1 - e0
        sp = np.zeros(ECP, np.int64)
        dp = np.zeros(ECP, np.int64)
        Wp = np.zeros((ECP, 32), np.float16)
        sp[:n] = src[e0:e1]
        dp[:n] = dst[e0:e1]
        Wp[:n] = W[e0:e1]
        # the gather ucode drops trailing-negative (int16) indices, so the
        # last edge of every 2048-edge chunk must have both ids < 32768;
        # swap such an edge into the last slot (undone on output)
        perm = np.arange(ECP)
        ok = (sp < 32768) & (dp < 32768)
        for base in range(0, ECP, CHUNK):
            last = base + CHUNK - 1
            if not ok[last]:
                j = base + int(np.argmax(ok[base:last]))
                for arr in (sp, dp, perm):
                    arr[last], arr[j] = arr[j], arr[last]
                Wp[[last, j]] = Wp[[j, last]]
        perms.append(perm)
        # per supertile, per chunk q: CPAD int16 ids at partitions
        # [32*(q%NQ_ACTIVE), +32) col block q//NQ_ACTIVE, ordinal i at
        # (+i%16, i//16), replicated to the pair's 2nd slab
        NQA = NQ_ACTIVE
        NBLK = NQ // NQA
        ids = np.empty((NTS, NQ, CPAD), np.int64)
        ids[:, 0::2] = sp.reshape(NTS, NQ // 2, CHUNK)
        ids[:, 1::2] = dp.reshape(NTS, NQ // 2, CHUNK)
        ids16 = ids.astype(np.int16)
        idxa = np.zeros((NTS, 4, 2, 16, NBLK, CPAD // 16), np.int16)
        for q in range(NQ):
            idxa[:, q % NQA, 0, ii % 16, q // NQA, ii // 16] = ids16[:, q]
        idxa[:, :, 1] = idxa[:, :, 0]
        idxa = idxa.reshape(NTS, P, NBLK * (CPAD // 16))
        # wt[t, d*32+c, r] = W[t*TSUB + r, c]
        wta = np.ascontiguousarray(
            np.tile(Wp.reshape(NTS, TSUB, 32).transpose(0, 2, 1), (1, 3, 1)))
        in_maps.append({
            "xnb": xnb,
            "idx": np.ascontiguousarray(idxa),
            "wt": wta,
            "mab": mab, "mbb": mbb,
        })
        spans.append((e0, e1))
    return in_maps, spans, perms, NTS, E


def kernel(xn, xe_src, xe_dst, W, M1, M2):
    in_maps, spans, perms, NTS, E = _prep_inputs(xn, xe_src, xe_dst, W, M1, M2)
    nc = _build_kernel(NTS, max(e1 - e0 for e0, e1 in spans))

    kw = {}
    if TRACE:
        import concourse.bass_utils as bu
        bu.upload_artifacts = lambda d: "skipped-local"
        kw = dict(trace=True, trace_cores=[0])
    res = run_bass_kernel_spmd(nc, in_maps, list(range(NCORES)), **kw)
    LAST_RESULTS["exec_time_ns"] = res.exec_time_ns
    LAST_RESULTS["mean_exec_time_ns"] = res.mean_exec_time_ns
    LAST_RESULTS["profile_json"] = res.profile_json
    LAST_RESULTS["instructions_and_trace"] = res.instructions_and_trace

    outp = np.empty((E, 3, 32), np.float32)
    for c in range(NCORES):
        e0, e1 = spans[c]
        n = e1 - e0
        # device layout [NTS, 96, TSUB] channel-major -> edge-major rows,
        # un-permuting the host-side chunk-tail swaps
        rows = res.results[c]["out"].transpose(0, 2, 1).reshape(-1, CH)
        perm = perms[c]
        valid = perm < n
        outp[e0 + perm[valid]] = (
            rows[valid].astype(np.float32).reshape(-1, 3, 32))
    return outp


# revision 47
# speedup vs baseline: 1.0301x; 1.0301x over previous
"""Trainium2 Bass kernel for nn_NodesToEdges (gnn message passing).

kernel(**inputs) takes FULL inputs, shards edges across 8 NeuronCores,
gathers node rows on-device, computes
  out[e] = 0.5*(W[e]*(xs-xd)) @ M1 + 0.25*(W[e]*(xs+xd)) @ M2
         = (W[e]*xs) @ Ma + (W[e]*xd) @ Mb,   Ma=.5*M1+.25*M2, Mb=.25*M2-.5*M1
and returns the FULL [E, 3, 32] f32 output.

v5 design (fp16 on device):
- Edges are processed in supertiles of TSUB=4096. The per-supertile
  node-row fetch (src+dst) uses the dma_gather ucode (non-transpose
  mode: index ordinal i lands at partition i%128, block i//128).
  Descriptor generation runs on the GPSIMD Q7 cluster at ~8ns/index
  and is the kernel bottleneck, but each gather instruction executes
  on one Q7 core PAIR selected by queue_num — so the fetch is split
  into 4 chunks issued on queues 0..3, which generate descriptors
  concurrently (4x). (The ucode's transposing gather mode would avoid
  the PE transposes below, but concurrent transposing gathers corrupt
  each other in the shared XBAR unit, so gathers run row-major and
  the PE — which has plenty of headroom here — transposes.)
- dma_gather indices are int16; node ids up to 50000 exceed the int16
  range, so the node table is laid out twice in one DRAM tensor and
  the gather base points at row 65536: the ucode's signed index
  addressing maps wrapped-negative ids to the right rows. Node rows
  sit at a 256-byte stride (ucode requirement) but only the real 192
  bytes are gathered. The host guarantees every chunk ends with an
  int16-positive id (the ucode drops trailing-negative indices).
- Per 512-edge group: the W-multiply runs on the DVE against a
  host-pretransposed W tile; the mix is 2 N=512 matmuls with the
  constant mixing matrices stationary, accumulating in PSUM; the
  Activation engine downcasts PSUM into the fp16 output tile. Output
  leaves the device channel-major; the host un-transposes and upcasts.
"""
import os
import sys

for p in ("/opt/trn_rl_repo", "/root/.axon_site/_ro/trn_rl_repo"):
    if os.path.isdir(p) and p not in sys.path:
        sys.path.append(p)
os.environ.setdefault("JAX_PLATFORMS", "axon")

import numpy as np
import concourse.bass as bass
import concourse.bacc as bacc
import concourse.mybir as mybir
from concourse import tile
from concourse.bass_utils import run_bass_kernel_spmd
from concourse.masks import make_identity

F32 = mybir.dt.float32
F16 = mybir.dt.float16
I16 = mybir.dt.int16
P = 128
CH = 96          # 3 vec dims x 32 channels
CHP = 128        # node row padded to 256B gather granularity
NCORES = 8
TSUB = 4096      # edges per supertile
G = TSUB // 512  # 512-edge groups per supertile
NQ = 4           # gather chunks
NQ_ACTIVE = 4  # distinct SWDGE queues (Q7 core pairs) used
CHUNK = 2 * TSUB // NQ        # ids per queue chunk (host guarantees the
CPAD = CHUNK                  # last id of each chunk is int16-positive)
NIDX = NQ * CPAD              # gathered columns per supertile
BIG = 98304      # wraparound node table rows (3 * 32768)
BASE = 65536     # gather base row inside the wraparound table

TRACE = False
LAST_RESULTS = {}


def _dma_gather_raw(eng, out_ap, in_ap, idxs_ap, num_idxs, elem_size,
                    elem_step, queue_num):
    """bass.dma_gather for the non-transpose HBM-source case, minus the
    256-byte elem_size restriction (that is an XBAR/transpose-mode
    constraint; the row-major ucode path only needs the row STRIDE to be
    a 256-byte multiple). elem_size=96 fp16 gathers the real 192 bytes
    of each 256-byte-strided node row."""
    import concourse.ap_utils as ap_utils
    assert idxs_ap.dtype == I16
    assert in_ap.dtype == out_ap.dtype
    assert ap_utils.ap_is_contiguous(in_ap.ap[1:])
    assert ap_utils.ap_is_contiguous(out_ap.ap[1:])
    assert ap_utils.ap_is_contiguous(idxs_ap.ap[1:])
    assert in_ap.ap[-1][1] == out_ap.ap[-1][1] == elem_size
    assert out_ap.ap[0][1] * out_ap.ap[1][1] == -(-num_idxs // P) * P
    assert in_ap.ap[0][0] == elem_step
    stride_bytes = elem_step * mybir.dt.size(in_ap.dtype)
    _in_ap = eng.lower_ap_dma(in_ap, for_custom_bir_dma=True)
    _idxs_ap = eng.lower_ap(idxs_ap)
    _out_ap = eng.lower_ap(out_ap)
    return eng.add_instruction(
        mybir.InstDMAGatherAnt(
            name=eng.bass.get_next_instruction_name(),
            ins=[*_in_ap, _idxs_ap,
                 eng.lower_val_access(eng.to_reg(num_idxs))],
            outs=[_out_ap],
            transpose=False,
            num_idxs=num_idxs,
            elem_size=elem_size,
            stride_bytes_256=stride_bytes // 256,
            gen_mode=0,
            single_packet=False,
            queue_num=queue_num,
            sbuf_tokens_per_rank=0,
            sbuf_free_dim_per_rank=0,
            sbuf_free_dim_pad_per_rank=0,
            sbuf_byte_offset=0,
        ))


def _build_kernel(NTS, n_real, n_devices=NCORES):
    nc = bacc.Bacc("TRN2", target_bir_lowering=False, debug=False,
                   num_devices=n_devices, num_swdge_queues=NQ)
    NBLK = NQ // NQ_ACTIVE
    xnb = nc.declare_dram_parameter("xnb", [BIG, CHP], F16, isOutput=False)
    idx = nc.declare_dram_parameter("idx", [NTS, P, NBLK * (CPAD // 16)], I16,
                                    isOutput=False)
    wt = nc.declare_dram_parameter("wt", [NTS, CH, TSUB], F16, isOutput=False)
    mab = nc.declare_dram_parameter("mab", [CH, CH], F16, isOutput=False)
    mbb = nc.declare_dram_parameter("mbb", [CH, CH], F16, isOutput=False)
    out = nc.declare_dram_parameter("out", [NTS, CH, TSUB], F16, isOutput=True)

    with tile.TileContext(nc) as tc:
        with (
            tc.tile_pool(name="const", bufs=1) as cp,
            tc.tile_pool(name="sb", bufs=3) as sb,
            tc.tile_pool(name="uv", bufs=6) as uv,
            tc.tile_pool(name="ps", bufs=2, space="PSUM") as ps,
            tc.tile_pool(name="po", bufs=3, space="PSUM") as po,
        ):
            for t in range(NTS):
                # chunk q's ids live in partitions [32*(q%NQ_ACTIVE), +32),
                # column block q//NQ_ACTIVE
                si = sb.tile([P, NBLK * (CPAD // 16)], I16, tag="si")
                nc.sync.dma_start(out=si[:], in_=idx[t])
                wtt = sb.tile([CH, TSUB], F16, tag="wt")
                nc.sync.dma_start(out=wtt[:], in_=wt[t])

                # 4 concurrent row-major gathers (one per Q7 pair):
                # chunk q, ordinal i -> xg[i%128, (q*BLKS + i//128)*96 : +96]
                CW = (CPAD // P) * CH
                xg = sb.tile([P, NQ * CW], F16, tag="xg")
                W16 = CPAD // 16
                for q in range(NQ):
                    # real edges in this chunk (pad-edge outputs are
                    # discarded on host, so skip gathering them)
                    real = max(0, min(n_real - t * TSUB - (q % 2) * CHUNK,
                                      CHUNK))
                    nidx = -(-real // P) * P
                    if nidx == 0:
                        continue
                    blk = q // NQ_ACTIVE
                    _dma_gather_raw(
                        nc.gpsimd,
                        out_ap=xg[:, q * CW:q * CW + (nidx // P) * CH]
                            .rearrange("p (b n) -> p b n", n=CH),
                        in_ap=xnb[BASE:, :CH],
                        idxs_ap=si[:, blk * W16:(blk + 1) * W16],
                        num_idxs=nidx, elem_size=CH, elem_step=CHP,
                        queue_num=q % NQ_ACTIVE)

                if t == 0:
                    # constants issued after supertile 0's gathers so the
                    # Q7 pipeline starts immediately; they are ready long
                    # before the first transpose needs them
                    ident = cp.tile([P, P], F16)
                    make_identity(nc, ident[:])
                    mab_t = cp.tile([CH, CH], F16)
                    nc.sync.dma_start(out=mab_t[:], in_=mab[:, :])
                    mbb_t = cp.tile([CH, CH], F16)
                    nc.sync.dma_start(out=mbb_t[:], in_=mbb[:, :])

                ot = sb.tile([CH, TSUB], F16, tag="ot")
                for g in range(G):
                    lo, hi = g * 512, (g + 1) * 512
                    # this group's src / dst blocks: groups 0..3 in chunks
                    # 0-1 (src), matching dst in chunks 2-3
                    sq, so = divmod(g * 512, CHUNK)
                    sbase = sq * CW + (so // P) * CH
                    dbase = (sq + 2) * CW + (so // P) * CH
                    xsT = ps.tile([CH, 512], F16, tag="xsT")
                    xdT = ps.tile([CH, 512], F16, tag="xdT")
                    for j in range(4):
                        nc.tensor.transpose(
                            out=xsT[:, j * P:(j + 1) * P],
                            in_=xg[:, sbase + j * CH:sbase + (j + 1) * CH],
                            identity=ident[:])
                    for j in range(4):
                        nc.tensor.transpose(
                            out=xdT[:, j * P:(j + 1) * P],
                            in_=xg[:, dbase + j * CH:dbase + (j + 1) * CH],
                            identity=ident[:])

                    uT = uv.tile([CH, 512], F16, tag="uT")
                    nc.vector.tensor_tensor(
                        out=uT[:], in0=xsT[:, :], in1=wtt[:, lo:hi],
                        op=mybir.AluOpType.mult)
                    vT = uv.tile([CH, 512], F16, tag="vT")
                    nc.vector.tensor_tensor(
                        out=vT[:], in0=xdT[:, :], in1=wtt[:, lo:hi],
                        op=mybir.AluOpType.mult)

                    oT = po.tile([CH, 512], F32, tag="oT")
                    nc.tensor.matmul(out=oT[:], lhsT=mab_t[:], rhs=uT[:],
                                     start=True, stop=False)
                    nc.tensor.matmul(out=oT[:], lhsT=mbb_t[:], rhs=vT[:],
                                     start=False, stop=True)
                    nc.scalar.copy(out=ot[:, lo:hi], in_=oT[:])

                nc.sync.dma_start(out=out[t], in_=ot[:])

    nc.compile()
    return nc


def _prep_inputs(xn, xe_src, xe_dst, W, M1, M2):
    E = int(xe_src.shape[0])
    nnodes = int(xn.shape[0])
    assert nnodes <= 65536

    src = np.asarray(xe_src).astype(np.int64)
    dst = np.asarray(xe_dst).astype(np.int64)
    W = np.asarray(W, dtype=np.float32)

    EC = -(-E // NCORES)           # edges per core (last may be short)
    ECP = -(-EC // TSUB) * TSUB    # padded per-core edge count
    NTS = ECP // TSUB

    M1d, M2d = np.asarray(M1, np.float64), np.asarray(M2, np.float64)
    Ma = 0.5 * M1d + 0.25 * M2d
    Mb = 0.25 * M2d - 0.5 * M1d
    mab = np.kron(np.eye(3), Ma).astype(np.float16)
    mbb = np.kron(np.eye(3), Mb).astype(np.float16)

    # wraparound node table, rows padded to 128 channels
    xn16 = np.asarray(xn, np.float32).reshape(nnodes, CH).astype(np.float16)
    xnb = np.zeros((BIG, CHP), np.float16)
    n_hi = max(0, nnodes - 32768)
    if n_hi:
        xnb[32768:32768 + n_hi, :CH] = xn16[32768:]
    xnb[BASE:BASE + min(nnodes, 32768), :CH] = xn16[:32768]

    in_maps, spans, perms = [], [], []
    ii = np.arange(CPAD)
    for c in range(NCORES):
        e0, e1 = c * EC, min(E, (c + 1) * EC)
        n = e1 - e0
        sp = np.zeros(ECP, np.int64)
        dp = np.zeros(ECP, np.int64)
        Wp = np.zeros((ECP, 32), np.float16)
        sp[:n] = src[e0:e1]
        dp[:n] = dst[e0:e1]
        Wp[:n] = W[e0:e1]
        # the gather ucode drops trailing-negative (int16) indices, so the
        # last edge of every 2048-edge chunk must have both ids < 32768;
        # swap such an edge into the last slot (undone on output)
        perm = np.arange(ECP)
        ok = (sp < 32768) & (dp < 32768)
        for base in range(0, ECP, CHUNK):
            last = base + CHUNK - 1
            if not ok[last]:
                j = base + int(np.argmax(ok[base:last]))
                for arr in (sp, dp, perm):
                    arr[last], arr[j] = arr[j], arr[last]
                Wp[[last, j]] = Wp[[j, last]]
        perms.append(perm)
        # per supertile, per chunk q: CPAD int16 ids at partitions
        # [32*(q%NQ_ACTIVE), +32) col block q//NQ_ACTIVE, ordinal i at
        # (+i%16, i//16), replicated to the pair's 2nd slab
        NQA = NQ_ACTIVE
        NBLK = NQ // NQA
        ids = np.empty((NTS, NQ, CPAD), np.int64)
        ids[:, 0:2] = sp.reshape(NTS, 2, CHUNK)
        ids[:, 2:4] = dp.reshape(NTS, 2, CHUNK)
        ids16 = ids.astype(np.int16)
        idxa = np.zeros((NTS, 4, 2, 16, NBLK, CPAD // 16), np.int16)
        for q in range(NQ):
            idxa[:, q % NQA, 0, ii % 16, q // NQA, ii // 16] = ids16[:, q]
        idxa[:, :, 1] = idxa[:, :, 0]
        idxa = idxa.reshape(NTS, P, NBLK * (CPAD // 16))
        # wt[t, d*32+c, r] = W[t*TSUB + r, c]
        wta = np.ascontiguousarray(
            np.tile(Wp.reshape(NTS, TSUB, 32).transpose(0, 2, 1), (1, 3, 1)))
        in_maps.append({
            "xnb": xnb,
            "idx": np.ascontiguousarray(idxa),
            "wt": wta,
            "mab": mab, "mbb": mbb,
        })
        spans.append((e0, e1))
    return in_maps, spans, perms, NTS, E


def kernel(xn, xe_src, xe_dst, W, M1, M2):
    in_maps, spans, perms, NTS, E = _prep_inputs(xn, xe_src, xe_dst, W, M1, M2)
    nc = _build_kernel(NTS, max(e1 - e0 for e0, e1 in spans))

    kw = {}
    if TRACE:
        import concourse.bass_utils as bu
        bu.upload_artifacts = lambda d: "skipped-local"
        kw = dict(trace=True, trace_cores=[0])
    res = run_bass_kernel_spmd(nc, in_maps, list(range(NCORES)), **kw)
    LAST_RESULTS["exec_time_ns"] = res.exec_time_ns
    LAST_RESULTS["mean_exec_time_ns"] = res.mean_exec_time_ns
    LAST_RESULTS["profile_json"] = res.profile_json
    LAST_RESULTS["instructions_and_trace"] = res.instructions_and_trace

    outp = np.empty((E, 3, 32), np.float32)
    for c in range(NCORES):
        e0, e1 = spans[c]
        n = e1 - e0
        # device layout [NTS, 96, TSUB] channel-major -> edge-major rows,
        # un-permuting the host-side chunk-tail swaps
        rows = res.results[c]["out"].transpose(0, 2, 1).reshape(-1, CH)
        perm = perms[c]
        valid = perm < n
        outp[e0 + perm[valid]] = (
            rows[valid].astype(np.float32).reshape(-1, 3, 32))
    return outp
